# revision 20
# baseline (speedup 1.0000x reference)
"""Trainium2 Bass kernel for nn_BilinearAttention2 (gnn_message_passing).

Math (per graph g, head h — where "head" h is a raw C-order reshape of the
[nA, D] block into [H, nA, HD], i.e. head h = 16 consecutive nodes reshaped):
  x1 = A @ W1.T + b1 ; x2 = B @ W2.T + b2
  X1 = x1[g].flat[h*4096:(h+1)*4096].reshape(128, 32)   (likewise X2)
  att[i,j]  = sum_k tanh(X1[i,k] * X2[j,k]) * q[k]
  b2a = softmax_i(mean_j att); a2b = softmax_j(mean_i att)
  A_p[g,h] = X1.T @ b2a ; B_p[g,h] = X2.T @ a2b
  out[g] = concat(A_p[g].flat, B_p[g].flat)    -> [G, 2D]

Sharding: data-parallel over graphs. 8 cores x 2 graphs each; weights
replicated. Each core processes its 16 (g,h) pairs as 4 "stacks" of 4 pairs:
partition dim = (pair-in-stack, k) = (4, 32) = 128, free dim = (i', j') = 16384.
One DVE broadcast-multiply + one ACT tanh per stack; the q-weighted k-sum is a
PE matmul with a block-diagonal q lhsT; both softmax reductions then act on the
small per-pair S matrices.
"""
import sys

sys.path.insert(0, "/opt/trn_rl_repo")

from contextlib import ExitStack

import numpy as np

import concourse.bass as bass
import concourse.bacc as bacc
import concourse.mybir as mybir
import concourse.tile as tile
from concourse.masks import make_identity

F32 = mybir.dt.float32
BF16 = mybir.dt.bfloat16

D = 256
H = 8
HD = 32
G = 16
NA = 128
NB = 128
NCORES = 8
GSH = G // NCORES          # graphs per core = 2
NPAIR = GSH * H            # 16 (g,h) pairs per core
SPP = 4                    # pairs per stack
NSTACK = NPAIR // SPP      # 4
NK = HD                    # 32
NJ = 128                   # nodes per head-view
FF = NJ * NJ               # 16384 free elems per stack


def build_kernel():
    nc = bacc.Bacc()
    a_d = nc.dram_tensor("A", [GSH * NA, D], F32, kind="ExternalInput")
    b_d = nc.dram_tensor("B", [GSH * NB, D], F32, kind="ExternalInput")
    w1_d = nc.dram_tensor("W1", [D, D], F32, kind="ExternalInput")
    w2_d = nc.dram_tensor("W2", [D, D], F32, kind="ExternalInput")
    b1_d = nc.dram_tensor("bias1", [1, D], F32, kind="ExternalInput")
    b2_d = nc.dram_tensor("bias2", [1, D], F32, kind="ExternalInput")
    q_d = nc.dram_tensor("q", [1, NK], F32, kind="ExternalInput")
    out_d = nc.dram_tensor("out", [GSH, 2 * D], F32, kind="ExternalOutput")

    # DRAM scratch
    x1_dram = nc.dram_tensor("x1_scratch", [GSH * NA, D], F32)
    x2_dram = nc.dram_tensor("x2_scratch", [GSH * NB, D], F32)

    with tile.TileContext(nc) as tc, ExitStack() as ctx:
        cst = ctx.enter_context(tc.tile_pool(name="cst", bufs=1))
        sbin = ctx.enter_context(tc.tile_pool(name="sbin", bufs=1))
        sbt = ctx.enter_context(tc.tile_pool(name="sbt", bufs=1))
        big = ctx.enter_context(tc.tile_pool(name="big", bufs=2))
        fold1 = ctx.enter_context(tc.tile_pool(name="fold1", bufs=1))
        sm = ctx.enter_context(tc.tile_pool(name="sm", bufs=2))
        pst = ctx.enter_context(tc.tile_pool(name="pst", bufs=2, space="PSUM"))
        psx = ctx.enter_context(tc.tile_pool(name="psx", bufs=2, space="PSUM"))
        pss = ctx.enter_context(tc.tile_pool(name="pss", bufs=2, space="PSUM"))
        ps1 = ctx.enter_context(tc.tile_pool(name="ps1", bufs=1, space="PSUM"))

        ident = cst.tile([128, 128], F32)
        make_identity(nc, ident[:])
        ones1 = cst.tile([1, 128], F32)
        nc.vector.memset(ones1[:], 1.0)
        onescol = cst.tile([128, 1], F32)
        nc.vector.memset(onescol[:], 1.0)

        # ---- qdiag [128, SPP]: qdiag[(pp,k), pp'] = q[k] * (pp == pp') ----
        q_sb = cst.tile([1, NK], F32)
        nc.sync.dma_start(q_sb[:], q_d[:])
        q_bf = cst.tile([1, NK], BF16)
        nc.vector.tensor_copy(q_bf[:], q_sb[:])
        qdiag = cst.tile([128, SPP], BF16)
        nc.vector.memset(qdiag[:], 0.0)
        for pp in range(SPP):
            nc.sync.dma_start(qdiag[pp * NK:(pp + 1) * NK, pp:pp + 1], q_bf[:])

        # ---- load inputs, transpose A,B,W1,W2 via PE ----
        def load_and_transpose(src_d, name):
            t_sb = []
            for c in range(2):
                tt = sbin.tile([128, D], F32, tag=f"{name}T{c}")
                t_sb.append(tt)
            for r in range(2):
                blk = sbin.tile([128, D], F32, tag=f"{name}ld")
                nc.sync.dma_start(blk[:], src_d[r * 128:(r + 1) * 128, :])
                for c in range(2):
                    tp = pst.tile([128, 128], F32, tag="tr")
                    nc.tensor.transpose(tp[:], blk[:, c * 128:(c + 1) * 128], ident[:])
                    nc.vector.tensor_copy(t_sb[c][:, r * 128:(r + 1) * 128], tp[:])
            return t_sb  # [c][128, 256] = transposed (contraction dim on partitions)

        at = load_and_transpose(a_d, "A")
        bt = load_and_transpose(b_d, "B")
        w1t = load_and_transpose(w1_d, "W1")
        w2t = load_and_transpose(w2_d, "W2")
        b1_sb = sbin.tile([1, D], F32)
        nc.sync.dma_start(b1_sb[:], b1_d[:])
        b2_sb = sbin.tile([1, D], F32)
        nc.sync.dma_start(b2_sb[:], b2_d[:])

        # ---- x1 = A@W1.T + b1, x2 = B@W2.T + b2 ; PSUM -> DRAM scratch ----
        for (xt, wt, bb, xd) in ((at, w1t, b1_sb, x1_dram), (bt, w2t, b2_sb, x2_dram)):
            for g in range(GSH):
                xp = psx.tile([128, D], F32, tag="xmm")
                nc.tensor.matmul(xp[:], xt[0][:, g * 128:(g + 1) * 128], wt[0][:], start=True, stop=False)
                nc.tensor.matmul(xp[:], xt[1][:, g * 128:(g + 1) * 128], wt[1][:], start=False, stop=False)
                nc.tensor.matmul(xp[:], ones1[0:1, :], bb[:], start=False, stop=True)
                xs = sbin.tile([128, D], F32, tag="xsb")
                nc.vector.tensor_copy(xs[:], xp[:])
                nc.sync.dma_start(xd[g * 128:(g + 1) * 128, :], xs[:])

        # ---- per stack: gather X1f/X2f [i', (pp,k)] f32 from DRAM scratch ----
        # pair pp of stack s: g = s // 2, h = (s % 2) * 4 + pp
        # X1f[(n ss), (pp k)] = x1[128 g + 16 h + n, 32 ss + k]
        x1f = []
        x2f = []
        x1t = []
        x2t = []
        for s in range(NSTACK):
            g, hq = s // 2, s % 2
            for (xd, fl, tl, nm) in ((x1_dram, x1f, x1t, "x1"), (x2_dram, x2f, x2t, "x2")):
                xf = sbt.tile([128, 128], F32, tag=f"{nm}f{s}")
                src = xd[:].rearrange("(g2 hq pp n) (ss k) -> g2 hq n ss pp k", g2=GSH, hq=2, pp=SPP, ss=H)
                nc.sync.dma_start(xf[:], src[g, hq])
                fl.append(xf)
                tp = pst.tile([128, 128], F32, tag="tr")
                nc.tensor.transpose(tp[:], xf[:], ident[:])
                xtb = sbt.tile([128, 128], BF16, tag=f"{nm}t{s}")
                nc.vector.tensor_copy(xtb[:], tp[:])
                tl.append(xtb)  # [(pp,k), i'] bf16

        # ---- main loop over stacks ----
        logits_a = sm.tile([NPAIR, NJ], F32, tag="la")
        logits_b = sm.tile([NPAIR, NJ], F32, tag="lb")
        for s in range(NSTACK):
            p4 = big.tile([128, FF], BF16, tag="p4")
            in0 = x1t[s][:].unsqueeze(2).broadcast_to([128, NJ, NJ])  # [(pp,k), i', j'<-bcast]
            in1 = x2t[s][:].unsqueeze(1).broadcast_to([128, NJ, NJ])  # [(pp,k), i'<-bcast, j']
            nc.vector.tensor_tensor(p4[:].rearrange("p (i j) -> p i j", j=NJ), in0, in1,
                                    op=mybir.AluOpType.mult)
            t4 = big.tile([128, FF], BF16, tag="t4")
            nc.scalar.activation(t4[:], p4[:], mybir.ActivationFunctionType.Tanh)

            # --- a2b: accumulate i'-quads on PE with q-block-diag lhsT ---
            # psum[pp', (iq, j')] = sum_ch sum_{(pp,k)} qdiag[(pp,k),pp'] T4[(pp,k), (4ch+iq, j')]
            a2b_ps = pss.tile([SPP, 4 * NJ], F32, tag="a2b")
            nch = FF // (4 * NJ)
            for ch in range(nch):
                nc.tensor.matmul(a2b_ps[:], qdiag[:], t4[:, ch * 4 * NJ:(ch + 1) * 4 * NJ],
                                 start=(ch == 0), stop=(ch == nch - 1))
            a2b_t = sm.tile([SPP, NJ], F32, tag="a2bt")
            nc.vector.tensor_reduce(
                a2b_t[:], a2b_ps[:].rearrange("p (iq j) -> p j iq", iq=4),
                axis=mybir.AxisListType.X, op=mybir.AluOpType.add)
            nc.sync.dma_start(logits_b[s * SPP:(s + 1) * SPP, :], a2b_t[:])

            # --- b2a: log-fold T4 over j' (GPS first fold, DVE rest), then qdiag mm ---
            v = t4[:].rearrange("p (ij2 two) -> p ij2 two", two=2)
            m1 = fold1.tile([128, FF // 2], BF16, tag="m1")
            nc.gpsimd.tensor_tensor(m1[:], v[:, :, 0:1].squeeze(2), v[:, :, 1:2].squeeze(2),
                                    op=mybir.AluOpType.add)
            # f32 fold chain in one segmented scratch: widths 4096,2048,1024,512,256
            msc = fold1.tile([128, 7936], F32, tag="msc")
            prev_ap, off = m1[:], 0
            for width in (4096, 2048, 1024, 512, 256):
                seg = msc[:, off:off + width]
                pv = prev_ap.rearrange("p (ij2 two) -> p ij2 two", two=2)
                nc.gpsimd.tensor_tensor(seg, pv[:, :, 0:1].squeeze(2), pv[:, :, 1:2].squeeze(2),
                                        op=mybir.AluOpType.add)
                prev_ap, off = seg, off + width
            cj4 = sm.tile([128, NJ], BF16, tag="cj4")
            pv = prev_ap.rearrange("p (ij2 two) -> p ij2 two", two=2)
            nc.gpsimd.tensor_tensor(cj4[:], pv[:, :, 0:1].squeeze(2), pv[:, :, 1:2].squeeze(2),
                                    op=mybir.AluOpType.add)
            b2a_ps = ps1.tile([SPP, NJ], F32, tag="b2a")
            nc.tensor.matmul(b2a_ps[:], qdiag[:], cj4[:], start=True, stop=True)
            b2a_t = sm.tile([SPP, NJ], F32, tag="b2at")
            nc.vector.tensor_copy(b2a_t[:], b2a_ps[:])
            nc.sync.dma_start(logits_a[s * SPP:(s + 1) * SPP, :], b2a_t[:])

        # ---- batched softmax over free dim (exp(x/128 - max/128) trick) ----
        def softmax16(lg, nm):
            mx = sm.tile([NPAIR, 1], F32, tag=f"mx{nm}")
            nc.vector.tensor_reduce(mx[:], lg[:], axis=mybir.AxisListType.X, op=mybir.AluOpType.max)
            nmx = sm.tile([NPAIR, 1], F32, tag=f"nmx{nm}")
            nc.vector.tensor_scalar_mul(nmx[:], mx[:], -1.0 / NJ)
            ex = sm.tile([NPAIR, NJ], F32, tag=f"ex{nm}")
            nc.scalar.activation(ex[:], lg[:], mybir.ActivationFunctionType.Exp,
                                 bias=nmx[:], scale=1.0 / NJ)
            sme = sm.tile([NPAIR, 1], F32, tag=f"sm{nm}")
            nc.vector.tensor_reduce(sme[:], ex[:], axis=mybir.AxisListType.X, op=mybir.AluOpType.add)
            rcp = sm.tile([NPAIR, 1], F32, tag=f"rc{nm}")
            nc.vector.reciprocal(rcp[:], sme[:])
            pr = sm.tile([NPAIR, NJ], F32, tag=f"pr{nm}")
            nc.vector.tensor_scalar_mul(pr[:], ex[:], rcp[:])
            return pr

        probs_a = softmax16(logits_a, "a")
        probs_b = softmax16(logits_b, "b")

        # transpose probs -> [i', pair]
        def transpose_probs(pr, nm):
            pp_ps = pst.tile([128, NPAIR], F32, tag="tr")
            nc.tensor.transpose(pp_ps[:], pr[:], ident[0:NPAIR, 0:NPAIR])
            pt = sm.tile([128, NPAIR], F32, tag=f"pt{nm}")
            nc.vector.tensor_copy(pt[:], pp_ps[:])
            return pt

    # probsT[:, pair] is the weight vector over i' (or j') for that pair
        probs_at = transpose_probs(probs_a, "a")
        probs_bt = transpose_probs(probs_b, "b")

        # ---- projections: per stack, out[(pp,k), pp'] = sum_i' X1f[i', (pp,k)] * probsT[i', pp'] ----
        for s in range(NSTACK):
            for (xf, pt, half) in ((x1f[s], probs_at, 0), (x2f[s], probs_bt, 1)):
                pj = ps1.tile([128, SPP], F32, tag="proj")
                nc.tensor.matmul(pj[:], xf[:], pt[:, s * SPP:(s + 1) * SPP], start=True, stop=True)
                pjs = sm.tile([128, SPP], F32, tag=f"projs{s}_{half}")
                nc.vector.tensor_copy(pjs[:], pj[:])
                for pp in range(SPP):
                    g, h = s // 2, (s % 2) * 4 + pp
                    eng = (nc.sync, nc.scalar, nc.gpsimd, nc.sync)[pp]
                    eng.dma_start(out_d[g:g + 1, half * D + h * HD: half * D + (h + 1) * HD],
                                  pjs[pp * NK:(pp + 1) * NK, pp:pp + 1])

    if not nc.is_finalized():
        nc.finalize()
    return nc


def shard_inputs(inputs):
    """Full inputs -> list of 8 per-core input maps."""
    A = np.asarray(inputs["A"], np.float32)
    B = np.asarray(inputs["B"], np.float32)
    maps = []
    for c in range(NCORES):
        maps.append({
            "A": np.ascontiguousarray(A[c * GSH * NA:(c + 1) * GSH * NA]),
            "B": np.ascontiguousarray(B[c * GSH * NB:(c + 1) * GSH * NB]),
            "W1": np.asarray(inputs["W1"], np.float32),
            "W2": np.asarray(inputs["W2"], np.float32),
            "bias1": np.asarray(inputs["bias1"], np.float32).reshape(1, D),
            "bias2": np.asarray(inputs["bias2"], np.float32).reshape(1, D),
            "q": np.asarray(inputs["q"], np.float32).reshape(1, NK),
        })
    return maps


_NC_CACHE = {}


def kernel(**inputs) -> np.ndarray:
    """Full (unsharded) inputs -> full [G, 2D] output, running on 8 cores."""
    from concourse.bass_utils import run_bass_kernel_spmd

    if "nc" not in _NC_CACHE:
        _NC_CACHE["nc"] = build_kernel()
    nc = _NC_CACHE["nc"]
    in_maps = shard_inputs(inputs)
    res = run_bass_kernel_spmd(nc, in_maps, core_ids=list(range(NCORES)))
    out = np.concatenate([res.results[c]["out"] for c in range(NCORES)], axis=0)
    return out.astype(np.float32)


if __name__ == "__main__":
    # CoreSim single-core debug: core 0 vs numpy reference
    from concourse.bass_interp import CoreSim

    rng = np.random.default_rng(0)
    scale = 1.0 / np.sqrt(D)
    full = {
        "A": rng.standard_normal((G * NA, D)).astype(np.float32),
        "B": rng.standard_normal((G * NB, D)).astype(np.float32),
        "W1": (rng.standard_normal((D, D)) * scale).astype(np.float32),
        "bias1": (rng.standard_normal(D) * scale).astype(np.float32),
        "W2": (rng.standard_normal((D, D)) * scale).astype(np.float32),
        "bias2": (rng.standard_normal(D) * scale).astype(np.float32),
        "q": (rng.standard_normal(HD) * scale).astype(np.float32),
    }

    def ref_core(m):
        x1 = m["A"] @ m["W1"].T + m["bias1"][0]
        x2 = m["B"] @ m["W2"].T + m["bias2"][0]
        x1 = x1.reshape(GSH, H, NA, HD)
        x2 = x2.reshape(GSH, H, NB, HD)
        att = np.einsum("ghijk,k->ghij", np.tanh(x1[:, :, :, None, :] * x2[:, :, None, :, :]), m["q"][0])

        def smax(v, ax):
            v = v - v.max(axis=ax, keepdims=True)
            e = np.exp(v)
            return e / e.sum(axis=ax, keepdims=True)

        b2a = smax(att.mean(axis=3), 2)
        a2b = smax(att.mean(axis=2), 2)
        A_p = np.einsum("ghik,ghi->ghk", x1, b2a).reshape(GSH, D)
        B_p = np.einsum("ghjk,ghj->ghk", x2, a2b).reshape(GSH, D)
        return np.concatenate([A_p, B_p], axis=1)

    nc = build_kernel()
    m0 = shard_inputs(full)[0]
    sim = CoreSim(nc)
    for k, v in m0.items():
        sim.tensor(k)[:] = v
    sim.simulate()
    got = sim.tensor("out").copy()
    want = ref_core(m0)
    err = np.abs(got - want).max() / np.abs(want).max()
    print("sim time:", sim.time, "ns")
    print("rel err:", err)
